# revision 24
# baseline (speedup 1.0000x reference)
"""Multi-head causal self-attention (B=4, S=2048, D=1024, H=16) on 8 TRN2 cores.

Sharding: core c handles batch b = c//2 and head-group hg = c%2 (8 of 16 heads).
Each core computes Q/K/V projections for its 8 heads, causal attention, and a
partial o-projection (columns of Wo.T for its head group); the host sums the
two partials per batch and transposes back.

Device layouts (per core):
  xt  [1024, 2048]  x[b].T                 (d on partitions, fp32r on SBUF)
  wq/wk/wv [1024, 512]  W[hg,:].T          (d on partitions, fp32r on SBUF)
  wo  [512, 1024] bf16  Wo[:, hg].T        (e_local on partitions)
  QT/KT [512, 2048] bf16                   (e_local on partitions)  "Q^T"
  V   16 x [128, 1024] bf16, per head interleaved [V_h(64) | ones(64)]
  E   exp(scores^T) bf16 tiles [k, q]
  CT  [512, 2048] bf16  ctx^T
  ot  [1024, 2048] f32  partial out^T

Attention per head pair: S^T[k,q] = K_h Q_h^T (bf16, two heads packed via
64-row PE tiling), exp on ACT (scale=1/8, additive -1e30 causal mask on the
diagonal 128-blocks), then one matmul per k-tile with lhsT=[V_h|ones] giving
ctx^T in rows 0:64 and the softmax denominator broadcast into rows 64:128 of
the same PSUM bank; normalize with reciprocal+multiply on DVE.  No softmax
max-subtraction: inputs are well-scaled so exp stays in fp32 range.

Scheduling: PE executes its queue in order, so projection / o-projection
matmul groups are interleaved (filler queue) into the ACT-paced attention
stretches to keep the PE busy.
"""
import sys
from collections import deque

if "/opt/trn_rl_repo" not in sys.path:
    sys.path.insert(0, "/opt/trn_rl_repo")

import numpy as np

D = 1024
S = 2048
B = 4
EL = 512            # local e width (8 heads x 64)
ND = D // 128       # 8 d-tiles
NE = EL // 128      # 4 local e-tiles
NS = S // 512       # 4 s/q blocks
NKT = S // 128      # 16 k-tiles
NEG = -1.0e30
SCALE = 0.125       # 1/sqrt(64)

_CACHE = {}


def _build(repeat=1):
    import concourse.tile as tile
    from concourse import bacc, mybir

    dt = mybir.dt
    f32, f32r, bf16 = dt.float32, dt.float32r, dt.bfloat16
    EXP = mybir.ActivationFunctionType.Exp

    nc = bacc.Bacc("TRN2", target_bir_lowering=False, debug=False)
    xt_d = nc.declare_dram_parameter("xt", [D, S], bf16, isOutput=False)
    wq_d = nc.declare_dram_parameter("wq", [D, EL], bf16, isOutput=False)
    wk_d = nc.declare_dram_parameter("wk", [D, EL], bf16, isOutput=False)
    wv_d = nc.declare_dram_parameter("wv", [D, EL], bf16, isOutput=False)
    wo_d = nc.declare_dram_parameter("wo", [EL, D], bf16, isOutput=False)
    mask_d = nc.declare_dram_parameter("mask", [128, 128], f32, isOutput=False)
    ot_d = nc.declare_dram_parameter("ot", [D, S], f32, isOutput=True)

    with tile.TileContext(nc) as tc:
        with tc.tile_pool(name="const", bufs=1) as constp, \
             tc.tile_pool(name="wts", bufs=1) as wtp, \
             tc.tile_pool(name="big", bufs=1) as bigp, \
             tc.tile_pool(name="xts", bufs=8) as xtp, \
             tc.tile_pool(name="ep", bufs=9) as epool, \
             tc.tile_pool(name="rp", bufs=2) as rpool, \
             tc.tile_pool(name="ost", bufs=3) as ostp, \
             tc.tile_pool(name="ps_proj", bufs=2, space="PSUM") as ps_proj, \
             tc.tile_pool(name="ps_s", bufs=2, space="PSUM") as ps_s, \
             tc.tile_pool(name="ps_ctx", bufs=2, space="PSUM") as ps_ctx:

            mask_t = constp.tile([128, 128], f32, tag="mask", name="mask_t")
            nc.sync.dma_start(mask_t[:], mask_d[:])

            for _rep in range(repeat):
                wq_t = [wtp.tile([128, EL], bf16, tag=f"wq{d}", name=f"wq{d}") for d in range(ND)]
                wk_t = [wtp.tile([128, EL], bf16, tag=f"wk{d}", name=f"wk{d}") for d in range(ND)]
                wv_t = [wtp.tile([128, EL], bf16, tag=f"wv{d}", name=f"wv{d}") for d in range(ND)]
                wo_t = [wtp.tile([128, D], bf16, tag=f"wo{e}", name=f"wo{e}") for e in range(NE)]
                QT = [bigp.tile([128, S], bf16, tag=f"qt{e}", name=f"qt{e}") for e in range(NE)]
                KT = [bigp.tile([128, S], bf16, tag=f"kt{e}", name=f"kt{e}") for e in range(NE)]
                V = [bigp.tile([128, 2 * EL], bf16, tag=f"v{k}", name=f"v{k}") for k in range(NKT)]
                CT = [bigp.tile([128, S], bf16, tag=f"ct{e}", name=f"ct{e}") for e in range(NE)]

                for k in range(NKT):
                    vv = V[k][:].rearrange("p (h t d) -> p h t d", t=2, d=64)
                    nc.gpsimd.memset(vv[:, :, 1, :], 1.0)

                xts = {}

                def load_xt(sb):
                    lst = []
                    for d in range(ND):
                        t = xtp.tile([128, 512], bf16, tag="xt", name="xt_t")
                        nc.sync.dma_start(t[:], xt_d[d * 128:(d + 1) * 128,
                                                     sb * 512:(sb + 1) * 512])
                        lst.append(t)
                    xts[sb] = lst

                # startup: interleave wq with the first x block so the first
                # projection group can begin after the first few transfers
                xts[0] = []
                for d in range(ND):
                    nc.sync.dma_start(wq_t[d][:], wq_d[d * 128:(d + 1) * 128, :])
                    t = xtp.tile([128, 512], bf16, tag="xt", name="xt_t")
                    nc.sync.dma_start(t[:], xt_d[d * 128:(d + 1) * 128, 0:512])
                    xts[0].append(t)
                for d in range(ND):
                    nc.sync.dma_start(wk_t[d][:], wk_d[d * 128:(d + 1) * 128, :])
                for d in range(ND):
                    nc.sync.dma_start(wv_t[d][:], wv_d[d * 128:(d + 1) * 128, :])
                for e in range(NE):
                    nc.sync.dma_start(wo_t[e][:], wo_d[e * 128:(e + 1) * 128, :])

                def qk_group(w_t, OUT, e, sb):
                    ps = ps_proj.tile([128, 512], f32, tag="psp", name="psp_t")
                    for d in range(ND):
                        nc.tensor.matmul(
                            ps[:], lhsT=w_t[d][:, e * 128:(e + 1) * 128],
                            rhs=xts[sb][d][:],
                            start=(d == 0), stop=(d == ND - 1))
                    nc.vector.tensor_copy(OUT[e][:, sb * 512:(sb + 1) * 512], ps[:])

                def v_group(sb, sc):
                    kt = sb * 4 + sc
                    ps = ps_proj.tile([128, 512], f32, tag="psp", name="psp_t")
                    for d in range(ND):
                        nc.tensor.matmul(
                            ps[:], lhsT=xts[sb][d][:, sc * 128:(sc + 1) * 128],
                            rhs=wv_t[d][:],
                            start=(d == 0), stop=(d == ND - 1))
                    vv = V[kt][:].rearrange("p (h t d) -> p h t d", t=2, d=64)
                    nc.vector.tensor_copy(vv[:, :, 0, :],
                                          ps[:].rearrange("p (h d) -> p h d", d=64))

                def oproj_group(eo, sbp, on_act=False):
                    ps = ps_proj.tile([128, 512], f32, tag="psp", name="psp_t")
                    for el in range(NE):
                        nc.tensor.matmul(
                            ps[:], lhsT=wo_t[el][:, eo * 128:(eo + 1) * 128],
                            rhs=CT[el][:, sbp * 512:(sbp + 1) * 512],
                            start=(el == 0), stop=(el == NE - 1))
                    ot_sb = ostp.tile([128, 512], f32, tag="ost", name="ot_sb")
                    if on_act:
                        nc.scalar.copy(ot_sb[:], ps[:])
                    else:
                        nc.vector.tensor_copy(ot_sb[:], ps[:])
                    nc.sync.dma_start(
                        ot_d[eo * 128:(eo + 1) * 128, sbp * 512:(sbp + 1) * 512],
                        ot_sb[:])

                def proj_fillers(sb, skip_q=False):
                    fns = []
                    for w_t, OUT in (((wk_t, KT),) if skip_q else ((wq_t, QT), (wk_t, KT))):
                        for e in range(NE):
                            fns.append(lambda w_t=w_t, OUT=OUT, e=e, sb=sb: qk_group(w_t, OUT, e, sb))
                    for sc in range(4):
                        fns.append(lambda sc=sc, sb=sb: v_group(sb, sc))
                    return [("proj", sb, f) for f in fns]

                filler = deque()

                def fill(n):
                    for _ in range(n):
                        if not filler:
                            return
                        filler.popleft()[2]()

                def flush(tag, key):
                    keep = deque()
                    while filler:
                        it = filler.popleft()
                        if it[0] == tag and it[1] == key:
                            it[2]()
                        else:
                            keep.append(it)
                    filler.extend(keep)

                # first Q projection d-major across four banks so the PE can
                # start as soon as each (wq, xt) tile pair lands
                q0ps = [ps_proj.tile([128, 512], f32, tag="psp", name="psp_t"),
                        ps_proj.tile([128, 512], f32, tag="psp", name="psp_t"),
                        ps_ctx.tile([128, 512], f32, tag="psc", name="psc_t"),
                        ps_ctx.tile([128, 512], f32, tag="psc", name="psc_t")]
                for d in range(ND):
                    for e in range(NE):
                        nc.tensor.matmul(
                            q0ps[e][:], lhsT=wq_t[d][:, e * 128:(e + 1) * 128],
                            rhs=xts[0][d][:],
                            start=(d == 0), stop=(d == ND - 1))
                for e in range(NE):
                    nc.vector.tensor_copy(QT[e][:, 0:512], q0ps[e][:])
                for it in proj_fillers(0, skip_q=True):
                    it[2]()

                for qb in range(NS):
                    if qb < NS - 1:
                        load_xt(qb + 1)
                        filler.extend(proj_fillers(qb + 1))
                    if qb == NS - 1:
                        for sbp in range(NS - 1):
                            filler.extend([("oproj", sbp,
                                            lambda eo=eo, sbp=sbp: oproj_group(eo, sbp))
                                           for eo in range(ND)])
                    nch = 2 * (qb + 1)
                    total_chunks = 4 * nch
                    fill_supply = len(filler)
                    fills_done = 0
                    chunk_idx = 0
                    for p in range(4):
                        cpA = ps_ctx.tile([128, 512], f32, tag="psc", name="psc_t")
                        cpB = ps_ctx.tile([128, 512], f32, tag="psc", name="psc_t")
                        cps = (cpA, cpB)
                        es = {}

                        def pv_chunk(ci, qb=qb, p=p, cps=cps, es=es):
                            for hp in (0, 1):
                                h = 2 * p + hp
                                cp = cps[hp]
                                e_t = es.pop((ci, hp))
                                for j in (0, 1):
                                    kt = 2 * ci + j
                                    dj = kt - 4 * qb
                                    v0 = 0 if dj < 1 else dj * 128
                                    nc.tensor.matmul(
                                        cp[:, v0:512],
                                        lhsT=V[kt][:, h * 128:(h + 1) * 128],
                                        rhs=e_t[:, j * 512 + v0:(j + 1) * 512],
                                        start=(kt == 0), stop=(kt == 4 * qb + 3))

                        pend = []
                        for ci in range(nch):
                            diag = (2 * ci + 1 - 4 * qb) >= 0
                            for hp in (0, 1):
                                hs = slice(hp * 64, (hp + 1) * 64)
                                sp = ps_s.tile([128, 1024], f32, tag="sps", name="sps_t")
                                for j in (0, 1):
                                    kt = 2 * ci + j
                                    dj = kt - 4 * qb
                                    c0 = 0 if dj < 1 else dj * 128
                                    nc.tensor.matmul(
                                        sp[:, j * 512 + c0:(j + 1) * 512],
                                        lhsT=KT[p][hs, kt * 128:(kt + 1) * 128],
                                        rhs=QT[p][hs, qb * 512 + c0:(qb + 1) * 512],
                                        start=True, stop=True,
                                        tile_position=(hp * 64, 0))
                                    if dj >= 0:
                                        dcol = j * 512 + dj * 128
                                        nc.vector.tensor_add(
                                            sp[:, dcol:dcol + 128],
                                            sp[:, dcol:dcol + 128],
                                            mask_t[:])
                                e_t = epool.tile([128, 1024], bf16, tag="e", name="e_t")
                                if not diag:
                                    nc.scalar.activation(e_t[:], sp[:], EXP, scale=SCALE)
                                else:
                                    for j in (0, 1):
                                        kt = 2 * ci + j
                                        dj = kt - 4 * qb
                                        c0 = 0 if dj < 1 else dj * 128
                                        nc.scalar.activation(
                                            e_t[:, j * 512 + c0:(j + 1) * 512],
                                            sp[:, j * 512 + c0:(j + 1) * 512],
                                            EXP, scale=SCALE)
                                es[(ci, hp)] = e_t
                            pend.append(ci)
                            chunk_idx += 1
                            want = (fill_supply * chunk_idx) // total_chunks
                            if want > fills_done:
                                fill(want - fills_done)
                                fills_done = want
                            if len(pend) > 3:
                                pv_chunk(pend.pop(0))
                        for ci in pend:
                            pv_chunk(ci)
                        for hp in (0, 1):
                            cp = cps[hp]
                            rc = rpool.tile([128, 512], f32, tag="recip", name="recip_t")
                            nc.vector.reciprocal(rc[0:64, :], cp[64:128, :])
                            nc.vector.tensor_mul(
                                CT[p][hp * 64:(hp + 1) * 64, qb * 512:(qb + 1) * 512],
                                cp[0:64, :], rc[0:64, :])
                    if qb < NS - 1:
                        flush("proj", qb + 1)

                filler.extend([("oproj", NS - 1,
                                lambda eo=eo: oproj_group(eo, NS - 1, on_act=(eo % 2 == 0)))
                               for eo in range(ND)])
                while filler:
                    filler.popleft()[2]()

    nc.compile()
    return nc


def _get(repeat=1):
    if repeat not in _CACHE:
        _CACHE[repeat] = _build(repeat)
    return _CACHE[repeat]


def _causal_mask():
    m = np.where(np.triu(np.ones((128, 128), dtype=bool)), 0.0, NEG)
    return m.astype(np.float32)


def _in_maps(x, Wq, Wk, Wv, Wo):
    import ml_dtypes

    mask = _causal_mask()
    maps = []
    for c in range(8):
        b, hg = c // 2, c % 2
        sl = slice(hg * EL, (hg + 1) * EL)
        maps.append({
            "xt": np.ascontiguousarray(x[b].T).astype(ml_dtypes.bfloat16),
            "wq": np.ascontiguousarray(Wq[sl, :].T).astype(ml_dtypes.bfloat16),
            "wk": np.ascontiguousarray(Wk[sl, :].T).astype(ml_dtypes.bfloat16),
            "wv": np.ascontiguousarray(Wv[sl, :].T).astype(ml_dtypes.bfloat16),
            "wo": np.ascontiguousarray(Wo[:, sl].T).astype(ml_dtypes.bfloat16),
            "mask": mask,
        })
    return maps


def kernel(x, Wq, Wk, Wv, Wo):
    from concourse.bass_utils import run_bass_kernel_spmd

    x = np.asarray(x, dtype=np.float32)
    nc = _get(1)
    maps = _in_maps(x, np.asarray(Wq), np.asarray(Wk), np.asarray(Wv), np.asarray(Wo))
    res = run_bass_kernel_spmd(nc, maps, core_ids=list(range(8)), trace=False)
    out = np.empty((B, S, D), dtype=np.float32)
    for b in range(B):
        out[b] = (res.results[2 * b]["ot"] + res.results[2 * b + 1]["ot"]).T
    return out


# revision 26
# speedup vs baseline: 1.6391x; 1.6391x over previous
"""Multi-head causal self-attention (B=4, S=2048, D=1024, H=16) on 8 TRN2 cores.

Sharding: core c handles batch b = c//2 and head-group hg = c%2 (8 of 16 heads).
Each core computes Q/K/V projections for its 8 heads, causal attention, and a
partial o-projection (columns of Wo.T for its head group); the host sums the
two partials per batch and transposes back.

Device layouts (per core):
  xt  [1024, 2048]  x[b].T  bf16           (d on partitions)
  wq/wk/wv [1024, 512]  W[hg,:].T  bf16    (d on partitions)
  wo  [512, 1024] bf16  Wo[:, hg].T        (e_local on partitions)
  QT/KT [512, 2048] bf16                   (e_local on partitions)  "Q^T"
  V   16 x [128, 1024] bf16, per head interleaved [V_h(64) | ones(64)]
  E   exp(scores^T) bf16 tiles [k, q]
  CT  [512, 2048] bf16  ctx^T
  ot  [1024, 2048] f32  partial out^T

Attention per head pair: S^T[k,q] = K_h Q_h^T (bf16, two heads packed via
64-row PE tiling), exp on ACT (scale=1/8, additive -1e30 causal mask on the
diagonal 128-blocks), then one matmul per k-tile with lhsT=[V_h|ones] giving
ctx^T in rows 0:64 and the softmax denominator broadcast into rows 64:128 of
the same PSUM bank; normalize with reciprocal+multiply on DVE.  No softmax
max-subtraction: inputs are well-scaled so exp stays in fp32 range.

Scheduling: PE executes its queue in order, so projection / o-projection
matmul groups are interleaved (filler queue) into the ACT-paced attention
stretches to keep the PE busy.
"""
import sys
from collections import deque

if "/opt/trn_rl_repo" not in sys.path:
    sys.path.insert(0, "/opt/trn_rl_repo")

import numpy as np

D = 1024
S = 2048
B = 4
EL = 512            # local e width (8 heads x 64)
ND = D // 128       # 8 d-tiles
NE = EL // 128      # 4 local e-tiles
NS = S // 512       # 4 s/q blocks
NKT = S // 128      # 16 k-tiles
NEG = -1.0e30
SCALE = 0.125       # 1/sqrt(64)

_CACHE = {}


def _build(repeat=1):
    import concourse.tile as tile
    from concourse import bacc, mybir

    dt = mybir.dt
    f32, f32r, bf16 = dt.float32, dt.float32r, dt.bfloat16
    EXP = mybir.ActivationFunctionType.Exp

    nc = bacc.Bacc("TRN2", target_bir_lowering=False, debug=False)
    xt_d = nc.declare_dram_parameter("xt", [D, S], bf16, isOutput=False)
    wq_d = nc.declare_dram_parameter("wq", [D, EL], bf16, isOutput=False)
    wk_d = nc.declare_dram_parameter("wk", [D, EL], bf16, isOutput=False)
    wv_d = nc.declare_dram_parameter("wv", [D, EL], bf16, isOutput=False)
    wo_d = nc.declare_dram_parameter("wo", [EL, D], bf16, isOutput=False)
    mask_d = nc.declare_dram_parameter("mask", [128, 128], f32, isOutput=False)
    ot_d = nc.declare_dram_parameter("ot", [D, S], f32, isOutput=True)

    with tile.TileContext(nc) as tc:
        with tc.tile_pool(name="const", bufs=1) as constp, \
             tc.tile_pool(name="wts", bufs=1) as wtp, \
             tc.tile_pool(name="big", bufs=1) as bigp, \
             tc.tile_pool(name="xts", bufs=8) as xtp, \
             tc.tile_pool(name="ep", bufs=9) as epool, \
             tc.tile_pool(name="rp", bufs=2) as rpool, \
             tc.tile_pool(name="ost", bufs=3) as ostp, \
             tc.tile_pool(name="ps_proj", bufs=2, space="PSUM") as ps_proj, \
             tc.tile_pool(name="ps_s", bufs=2, space="PSUM") as ps_s, \
             tc.tile_pool(name="ps_ctx", bufs=2, space="PSUM") as ps_ctx:

            mask_t = constp.tile([128, 128], f32, tag="mask", name="mask_t")
            nc.sync.dma_start(mask_t[:], mask_d[:])

            for _rep in range(repeat):
                wq_t = [wtp.tile([128, EL], bf16, tag=f"wq{d}", name=f"wq{d}") for d in range(ND)]
                wk_t = [wtp.tile([128, EL], bf16, tag=f"wk{d}", name=f"wk{d}") for d in range(ND)]
                wv_t = [wtp.tile([128, EL], bf16, tag=f"wv{d}", name=f"wv{d}") for d in range(ND)]
                wo_t = [wtp.tile([128, D], bf16, tag=f"wo{e}", name=f"wo{e}") for e in range(NE)]
                QT = [bigp.tile([128, S], bf16, tag=f"qt{e}", name=f"qt{e}") for e in range(NE)]
                KT = [bigp.tile([128, S], bf16, tag=f"kt{e}", name=f"kt{e}") for e in range(NE)]
                V = [bigp.tile([128, 2 * EL], bf16, tag=f"v{k}", name=f"v{k}") for k in range(NKT)]
                CT = [bigp.tile([128, S], bf16, tag=f"ct{e}", name=f"ct{e}") for e in range(NE)]

                for k in range(NKT):
                    vv = V[k][:].rearrange("p (h t d) -> p h t d", t=2, d=64)
                    nc.gpsimd.memset(vv[:, :, 1, :], 1.0)

                xts = {}

                def load_xt(sb):
                    lst = []
                    for d in range(ND):
                        t = xtp.tile([128, 512], bf16, tag="xt", name="xt_t")
                        nc.sync.dma_start(t[:], xt_d[d * 128:(d + 1) * 128,
                                                     sb * 512:(sb + 1) * 512])
                        lst.append(t)
                    xts[sb] = lst

                # startup: interleave wq with the first x block so the first
                # projection group can begin after the first few transfers
                xts[0] = []
                for d in range(ND):
                    nc.sync.dma_start(wq_t[d][:], wq_d[d * 128:(d + 1) * 128, :])
                    t = xtp.tile([128, 512], bf16, tag="xt", name="xt_t")
                    nc.sync.dma_start(t[:], xt_d[d * 128:(d + 1) * 128, 0:512])
                    xts[0].append(t)
                for d in range(ND):
                    nc.sync.dma_start(wk_t[d][:], wk_d[d * 128:(d + 1) * 128, :])
                for d in range(ND):
                    nc.sync.dma_start(wv_t[d][:], wv_d[d * 128:(d + 1) * 128, :])
                for e in range(NE):
                    nc.sync.dma_start(wo_t[e][:], wo_d[e * 128:(e + 1) * 128, :])

                def qk_group(w_t, OUT, e, sb):
                    ps = ps_proj.tile([128, 512], f32, tag="psp", name="psp_t")
                    for d in range(ND):
                        nc.tensor.matmul(
                            ps[:], lhsT=w_t[d][:, e * 128:(e + 1) * 128],
                            rhs=xts[sb][d][:],
                            start=(d == 0), stop=(d == ND - 1))
                    nc.vector.tensor_copy(OUT[e][:, sb * 512:(sb + 1) * 512], ps[:])

                def v_group(sb, sc):
                    kt = sb * 4 + sc
                    ps = ps_proj.tile([128, 512], f32, tag="psp", name="psp_t")
                    for d in range(ND):
                        nc.tensor.matmul(
                            ps[:], lhsT=xts[sb][d][:, sc * 128:(sc + 1) * 128],
                            rhs=wv_t[d][:],
                            start=(d == 0), stop=(d == ND - 1))
                    vv = V[kt][:].rearrange("p (h t d) -> p h t d", t=2, d=64)
                    nc.vector.tensor_copy(vv[:, :, 0, :],
                                          ps[:].rearrange("p (h d) -> p h d", d=64))

                def oproj_group(eo, sbp, on_act=False):
                    ps = ps_proj.tile([128, 512], f32, tag="psp", name="psp_t")
                    for el in range(NE):
                        nc.tensor.matmul(
                            ps[:], lhsT=wo_t[el][:, eo * 128:(eo + 1) * 128],
                            rhs=CT[el][:, sbp * 512:(sbp + 1) * 512],
                            start=(el == 0), stop=(el == NE - 1))
                    ot_sb = ostp.tile([128, 512], f32, tag="ost", name="ot_sb")
                    if on_act:
                        nc.scalar.copy(ot_sb[:], ps[:])
                    else:
                        nc.vector.tensor_copy(ot_sb[:], ps[:])
                    nc.sync.dma_start(
                        ot_d[eo * 128:(eo + 1) * 128, sbp * 512:(sbp + 1) * 512],
                        ot_sb[:])

                def proj_fillers(sb, skip_q=False):
                    fns = []
                    for w_t, OUT in (((wk_t, KT),) if skip_q else ((wq_t, QT), (wk_t, KT))):
                        for e in range(NE):
                            fns.append(lambda w_t=w_t, OUT=OUT, e=e, sb=sb: qk_group(w_t, OUT, e, sb))
                    for sc in range(4):
                        fns.append(lambda sc=sc, sb=sb: v_group(sb, sc))
                    return [("proj", sb, f) for f in fns]

                filler = deque()

                def fill(n):
                    for _ in range(n):
                        if not filler:
                            return
                        filler.popleft()[2]()

                def flush(tag, key):
                    keep = deque()
                    while filler:
                        it = filler.popleft()
                        if it[0] == tag and it[1] == key:
                            it[2]()
                        else:
                            keep.append(it)
                    filler.extend(keep)

                # first Q projection d-major across four banks so the PE can
                # start as soon as each (wq, xt) tile pair lands
                q0ps = [ps_proj.tile([128, 512], f32, tag="psp", name="psp_t"),
                        ps_proj.tile([128, 512], f32, tag="psp", name="psp_t"),
                        ps_ctx.tile([128, 512], f32, tag="psc", name="psc_t"),
                        ps_ctx.tile([128, 512], f32, tag="psc", name="psc_t")]
                for d in range(ND):
                    for e in range(NE):
                        nc.tensor.matmul(
                            q0ps[e][:], lhsT=wq_t[d][:, e * 128:(e + 1) * 128],
                            rhs=xts[0][d][:],
                            start=(d == 0), stop=(d == ND - 1))
                for e in range(NE):
                    nc.vector.tensor_copy(QT[e][:, 0:512], q0ps[e][:])
                k0ps = [ps_proj.tile([128, 512], f32, tag="psp", name="psp_t"),
                        ps_proj.tile([128, 512], f32, tag="psp", name="psp_t"),
                        ps_ctx.tile([128, 512], f32, tag="psc", name="psc_t"),
                        ps_ctx.tile([128, 512], f32, tag="psc", name="psc_t")]
                for d in range(ND):
                    for e in range(NE):
                        nc.tensor.matmul(
                            k0ps[e][:], lhsT=wk_t[d][:, e * 128:(e + 1) * 128],
                            rhs=xts[0][d][:],
                            start=(d == 0), stop=(d == ND - 1))
                for e in range(NE):
                    nc.vector.tensor_copy(KT[e][:, 0:512], k0ps[e][:])
                for sc in range(4):
                    v_group(0, sc)

                for qb in range(NS):
                    if qb < NS - 1:
                        load_xt(qb + 1)
                        filler.extend(proj_fillers(qb + 1))
                    if qb == NS - 1:
                        for sbp in range(NS - 1):
                            filler.extend([("oproj", sbp,
                                            lambda eo=eo, sbp=sbp: oproj_group(eo, sbp))
                                           for eo in range(ND)])
                    nch = 2 * (qb + 1)
                    total_chunks = 4 * nch
                    fill_supply = len(filler)
                    fills_done = 0
                    chunk_idx = 0
                    for p in range(4):
                        cpA = ps_ctx.tile([128, 512], f32, tag="psc", name="psc_t")
                        cpB = ps_ctx.tile([128, 512], f32, tag="psc", name="psc_t")
                        cps = (cpA, cpB)
                        es = {}

                        def pv_chunk(ci, qb=qb, p=p, cps=cps, es=es):
                            for hp in (0, 1):
                                h = 2 * p + hp
                                cp = cps[hp]
                                e_t = es.pop((ci, hp))
                                for j in (0, 1):
                                    kt = 2 * ci + j
                                    dj = kt - 4 * qb
                                    v0 = 0 if dj < 1 else dj * 128
                                    nc.tensor.matmul(
                                        cp[:, v0:512],
                                        lhsT=V[kt][:, h * 128:(h + 1) * 128],
                                        rhs=e_t[:, j * 512 + v0:(j + 1) * 512],
                                        start=(kt == 0), stop=(kt == 4 * qb + 3))

                        pend = []
                        for ci in range(nch):
                            diag = (2 * ci + 1 - 4 * qb) >= 0
                            for hp in (0, 1):
                                hs = slice(hp * 64, (hp + 1) * 64)
                                sp = ps_s.tile([128, 1024], f32, tag="sps", name="sps_t")
                                for j in (0, 1):
                                    kt = 2 * ci + j
                                    dj = kt - 4 * qb
                                    c0 = 0 if dj < 1 else dj * 128
                                    nc.tensor.matmul(
                                        sp[:, j * 512 + c0:(j + 1) * 512],
                                        lhsT=KT[p][hs, kt * 128:(kt + 1) * 128],
                                        rhs=QT[p][hs, qb * 512 + c0:(qb + 1) * 512],
                                        start=True, stop=True,
                                        tile_position=(hp * 64, 0))
                                    if dj >= 0:
                                        dcol = j * 512 + dj * 128
                                        nc.vector.tensor_add(
                                            sp[:, dcol:dcol + 128],
                                            sp[:, dcol:dcol + 128],
                                            mask_t[:])
                                e_t = epool.tile([128, 1024], bf16, tag="e", name="e_t")
                                if not diag:
                                    nc.scalar.activation(e_t[:], sp[:], EXP, scale=SCALE)
                                else:
                                    for j in (0, 1):
                                        kt = 2 * ci + j
                                        dj = kt - 4 * qb
                                        c0 = 0 if dj < 1 else dj * 128
                                        nc.scalar.activation(
                                            e_t[:, j * 512 + c0:(j + 1) * 512],
                                            sp[:, j * 512 + c0:(j + 1) * 512],
                                            EXP, scale=SCALE)
                                es[(ci, hp)] = e_t
                            pend.append(ci)
                            chunk_idx += 1
                            want = (fill_supply * chunk_idx) // total_chunks
                            if want > fills_done:
                                fill(want - fills_done)
                                fills_done = want
                            if len(pend) > 3:
                                pv_chunk(pend.pop(0))
                        for ci in pend:
                            pv_chunk(ci)
                        for hp in (0, 1):
                            cp = cps[hp]
                            rc = rpool.tile([128, 512], f32, tag="recip", name="recip_t")
                            nc.vector.reciprocal(rc[0:64, :], cp[64:128, :])
                            nc.vector.tensor_mul(
                                CT[p][hp * 64:(hp + 1) * 64, qb * 512:(qb + 1) * 512],
                                cp[0:64, :], rc[0:64, :])
                    if qb < NS - 1:
                        flush("proj", qb + 1)

                filler.extend([("oproj", NS - 1,
                                lambda eo=eo: oproj_group(eo, NS - 1, on_act=(eo % 2 == 0)))
                               for eo in range(ND)])
                while filler:
                    filler.popleft()[2]()

    nc.compile()
    return nc


def _get(repeat=1):
    if repeat not in _CACHE:
        _CACHE[repeat] = _build(repeat)
    return _CACHE[repeat]


def _causal_mask():
    m = np.where(np.triu(np.ones((128, 128), dtype=bool)), 0.0, NEG)
    return m.astype(np.float32)


def _in_maps(x, Wq, Wk, Wv, Wo):
    import ml_dtypes

    mask = _causal_mask()
    maps = []
    for c in range(8):
        b, hg = c // 2, c % 2
        sl = slice(hg * EL, (hg + 1) * EL)
        maps.append({
            "xt": np.ascontiguousarray(x[b].T).astype(ml_dtypes.bfloat16),
            "wq": np.ascontiguousarray(Wq[sl, :].T).astype(ml_dtypes.bfloat16),
            "wk": np.ascontiguousarray(Wk[sl, :].T).astype(ml_dtypes.bfloat16),
            "wv": np.ascontiguousarray(Wv[sl, :].T).astype(ml_dtypes.bfloat16),
            "wo": np.ascontiguousarray(Wo[:, sl].T).astype(ml_dtypes.bfloat16),
            "mask": mask,
        })
    return maps


def kernel(x, Wq, Wk, Wv, Wo):
    from concourse.bass_utils import run_bass_kernel_spmd

    x = np.asarray(x, dtype=np.float32)
    nc = _get(1)
    maps = _in_maps(x, np.asarray(Wq), np.asarray(Wk), np.asarray(Wv), np.asarray(Wo))
    res = run_bass_kernel_spmd(nc, maps, core_ids=list(range(8)), trace=False)
    out = np.empty((B, S, D), dtype=np.float32)
    for b in range(B):
        out[b] = (res.results[2 * b]["ot"] + res.results[2 * b + 1]["ot"]).T
    return out
